# revision 6
# baseline (speedup 1.0000x reference)
"""Causal self-attention Bass kernel for TRN2, 8 NeuronCores.

Sharding: data-parallel over batch (B=4) x tensor-parallel over head halves
(2 groups of 8 heads) = 8 shards, Megatron-style. Each core computes its
batch's qkv projection for its 8 heads, causal attention, and a partial
output projection (its heads' rows of W_proj). The host sums the two
partials per batch and adds b_proj.

All matmul operands are fp16 (full-rate 1 cycle/row on the PE, fp32 PSUM
accumulation). Output partials are stored fp16 (halves DMA; host sums in
fp32).

Schedule (v2): the per-si chain is scores -> exp(ACT) -> PV, with ACT and
PE in near-lockstep. The emission software-pipelines each attention block
(scores si+2 ahead of PV si) and injects filler matmuls (V/QK projections
for later rounds, output-projection chunks for earlier rounds) between the
scores and PV of each si so the PE never waits on the exp. The causal mask
is applied as a -30000 bias matmul into PSUM on diagonal chunks before the
exp (fp16 underflow -> exact zeros), keeping DVE off the critical chain.

Layouts per core:
  xt   = x[b].T (fp16)                 (C=1024, T=2048), 32 [128,512] tiles
  wqk  = [Wq_half | Wk_half] (fp16)    (1024, 1024)
  wv   = Wv_half (fp16)                (1024, 512)
  wp   = W_proj[512*h2:+512, :] (fp16) (512, 1024)
  QT/KT tiles [128, 512] fp16: partitions = d + 64*(h%2) for head pair h//2
  V tiles [128, 8, 65] fp16: per s-chunk, 8 heads x (64 V cols + ones col)
  scores^T [s,t] 2-head row-packed (concurrent row-tiled MMs), diag cols
  clipped + triangle bias-masked -> ACT exp -> PV matmul M=65 -> O^T[d,t]
  + Z row in PSUM -> recip_approx_fast + gpsimd partition_broadcast ->
  normalized OCT (SBUF) -> proj: out[t, c] = sum_hd OCT[hd, t] * wp[hd, c]
"""

import math
import os
from collections import deque

import numpy as np

import concourse.bass as bass
import concourse.mybir as mybir
from concourse import bacc
from concourse.tile import TileContext

F32 = mybir.dt.float32
F16 = mybir.dt.float16

N_EMBD = 1024
N_HEAD = 16
D = 64
B = 4
T = 2048
N_CORES = 8
PAIRS = 4          # head pairs per core (8 heads)
TJ = T // 512      # 512-wide t super-chunks
SJ = T // 128      # 128-wide s chunks
SCALE = 1.0 / math.sqrt(D)
MASK_BIAS = -30000.0

_CACHE = {}


def _build():
    nc = bacc.Bacc()

    xt_d = nc.declare_dram_parameter("xt", [N_EMBD, T], F16, isOutput=False)
    wqk_d = nc.declare_dram_parameter("wqk", [N_EMBD, 1024], F16, isOutput=False)
    wv_d = nc.declare_dram_parameter("wv", [N_EMBD, 512], F16, isOutput=False)
    wp_d = nc.declare_dram_parameter("wp", [512, N_EMBD], F16, isOutput=False)
    bqk_d = nc.declare_dram_parameter("bqk", [128, 8], F32, isOutput=False)
    bv_d = nc.declare_dram_parameter("bv", [1, 512], F16, isOutput=False)
    out_d = nc.declare_dram_parameter("out_p", [T, N_EMBD], F16, isOutput=True)

    with TileContext(nc) as tc:
        with (
            tc.tile_pool(name="const", bufs=1) as cpool,
            tc.tile_pool(name="w", bufs=1) as wpool,
            tc.tile_pool(name="xt", bufs=1) as xpool,
            tc.tile_pool(name="qkt", bufs=1) as qkpool,
            tc.tile_pool(name="v", bufs=1) as vpool,
            tc.tile_pool(name="e", bufs=4) as epool,
            tc.tile_pool(name="octp", bufs=1) as octpool,
            tc.tile_pool(name="rz", bufs=2) as rzpool,
            tc.tile_pool(name="outp", bufs=4) as opool,
            tc.tile_pool(name="ps", bufs=2, space="PSUM") as pspool,
            tc.tile_pool(name="pv", bufs=3, space="PSUM") as pvpool,
            tc.tile_pool(name="fl", bufs=1, space="PSUM") as flpool,
        ):
            # ---- constants ----
            ones_f = cpool.tile([1, 128], F32, tag="ones_f")
            nc.vector.memset(ones_f, 1.0)
            ones_r = cpool.tile([1, 128], F16, tag="ones_r")
            nc.vector.tensor_copy(ones_r, ones_f)
            ones8 = cpool.tile([128, 8], F32, tag="ones8")
            nc.vector.memset(ones8, 1.0)
            bqk_t = cpool.tile([128, 8], F32, tag="bqk")
            nc.sync.dma_start(out=bqk_t, in_=bqk_d[:, :])
            bv_t = cpool.tile([1, 512], F16, tag="bv")
            nc.sync.dma_start(out=bv_t, in_=bv_d[:, :])

            # identity [128,128] fp16 (bias-matmul lhsT)
            ident = cpool.tile([128, 128], F16, tag="ident")
            nc.vector.memset(ident, 1.0)
            nc.gpsimd.affine_select(
                out=ident, in_=ident, compare_op=mybir.AluOpType.is_ge, fill=0.0,
                base=0, pattern=[[1, 128]], channel_multiplier=-1,
            )
            nc.gpsimd.affine_select(
                out=ident, in_=ident, compare_op=mybir.AluOpType.is_ge, fill=0.0,
                base=0, pattern=[[-1, 128]], channel_multiplier=1,
            )
            # btri[p, j] = 0 where j >= p else MASK_BIAS  (causal triangle)
            btri = cpool.tile([128, 128], F16, tag="btri")
            nc.vector.memset(btri, 0.0)
            nc.gpsimd.affine_select(
                out=btri, in_=btri, compare_op=mybir.AluOpType.is_ge,
                fill=MASK_BIAS, base=0, pattern=[[1, 128]], channel_multiplier=-1,
            )

            # ---- weight/x DMAs: wv+x on sync queue, wqk/wp on vector queue ----
            wqk = []
            wv = []
            wp = []
            XT = [[None] * TJ for _ in range(8)]  # [c][tj] -> [128, 512]
            for c in range(8):
                t = wpool.tile([128, 512], F16, tag=f"wv{c}")
                nc.sync.dma_start(out=t, in_=wv_d[128 * c : 128 * c + 128, :])
                wv.append(t)
                tx = xpool.tile([128, 512], F16, tag=f"x{c}_0")
                nc.sync.dma_start(out=tx, in_=xt_d[128 * c : 128 * c + 128, 0:512])
                XT[c][0] = tx
            for c in range(8):
                t = wpool.tile([128, 1024], F16, tag=f"wqk{c}")
                nc.scalar.dma_start(out=t, in_=wqk_d[128 * c : 128 * c + 128, :])
                wqk.append(t)
            for tj in range(1, TJ):
                for c in range(8):
                    tx = xpool.tile([128, 512], F16, tag=f"x{c}_{tj}")
                    nc.sync.dma_start(
                        out=tx,
                        in_=xt_d[128 * c : 128 * c + 128, 512 * tj : 512 * tj + 512],
                    )
                    XT[c][tj] = tx
            for p in range(PAIRS):
                t = wpool.tile([128, 1024], F16, tag=f"wp{p}")
                nc.scalar.dma_start(out=t, in_=wp_d[128 * p : 128 * p + 128, :])
                wp.append(t)

            # bvb = b_v broadcast to [128, 512] via K=1 matmul
            ps_bvb = flpool.tile([128, 512], F32, tag="f")
            nc.tensor.matmul(
                ps_bvb, lhsT=ones_r[0:1, :], rhs=bv_t, start=True, stop=True
            )
            bvb = cpool.tile([128, 512], F32, tag="bvb")
            nc.vector.tensor_copy(bvb, ps_bvb)

            QT = [[None] * TJ for _ in range(PAIRS)]
            KT = [[None] * TJ for _ in range(PAIRS)]
            V = [None] * SJ
            OCT = [[None] * TJ for _ in range(PAIRS)]

            # ---------- filler machinery ----------
            # Each filler item: (cost_ns, closure). Closures emit ONE matmul,
            # plus evictions when they complete a chunk.
            filler = deque()

            def v_chunk_items(s_idx):
                tj, sj = s_idx // 4, s_idx % 4
                state = {}

                def mk(c):
                    def emit():
                        if c == 0:
                            state["pv"] = flpool.tile([128, 512], F32, tag="f", name="flv")
                        nc.tensor.matmul(
                            state["pv"],
                            lhsT=XT[c][tj][:, 128 * sj : 128 * sj + 128],
                            rhs=wv[c],
                            start=(c == 0),
                            stop=(c == 7),
                        )
                        if c == 7:
                            vt = vpool.tile([128, 8, 65], F16, tag=f"v{s_idx}")
                            nc.vector.tensor_add(
                                vt[:, :, 0:64],
                                state["pv"].rearrange("p (h d) -> p h d", h=8),
                                bvb.rearrange("p (h d) -> p h d", h=8),
                            )
                            nc.vector.tensor_copy(
                                vt[:, :, 64:65],
                                ones8.rearrange("p (h o) -> p h o", h=8),
                            )
                            V[s_idx] = vt
                    return emit

                return [(213, mk(c)) for c in range(8)]

            def qk_chunk_items(tj, n):
                state = {}

                def mk(c):
                    def emit():
                        if c == 0:
                            state["ps"] = flpool.tile([128, 512], F32, tag="f", name="flqk")
                        nc.tensor.matmul(
                            state["ps"],
                            lhsT=wqk[c][:, 128 * n : 128 * n + 128],
                            rhs=XT[c][tj],
                            start=(c == 0),
                            stop=(c == 7),
                        )
                        if c == 7:
                            dst = qkpool.tile([128, 512], F16, tag=f"qk{n}_{tj}")
                            nc.vector.tensor_scalar_add(
                                dst, state["ps"], bqk_t[:, n : n + 1]
                            )
                            if n < 4:
                                QT[n][tj] = dst
                            else:
                                KT[n - 4][tj] = dst
                    return emit

                return [(213, mk(c)) for c in range(8)]

            def proj_chunk_items(t2, cj, evict_on_act=False):
                tcj, k = t2 // 4, t2 % 4
                state = {}

                def mk(pair):
                    def emit():
                        if pair == 0:
                            state["po"] = flpool.tile([128, 512], F32, tag="f", name="flpo")
                        nc.tensor.matmul(
                            state["po"],
                            lhsT=OCT[pair][tcj][:, 128 * k : 128 * k + 128],
                            rhs=wp[pair][:, 512 * cj : 512 * cj + 512],
                            start=(pair == 0),
                            stop=(pair == 3),
                        )
                        if pair == 3:
                            ot = opool.tile([128, 512], F16, tag="out")
                            if evict_on_act:
                                nc.scalar.copy(ot, state["po"])
                            else:
                                nc.vector.tensor_copy(ot, state["po"])
                            nc.gpsimd.dma_start(
                                out=out_d[
                                    128 * t2 : 128 * t2 + 128,
                                    512 * cj : 512 * cj + 512,
                                ],
                                in_=ot,
                            )
                    return emit

                return [(213, mk(pair)) for pair in range(PAIRS)]

            def drain_filler(n=None):
                cnt = len(filler) if n is None else min(n, len(filler))
                for _ in range(cnt):
                    _, emit = filler.popleft()
                    emit()

            def take_filler(budget_ns):
                """Emit filler until ~budget_ns of PE work injected."""
                spent = 0
                while filler and spent < budget_ns:
                    cost, emit = filler.popleft()
                    emit()
                    spent += cost
                return spent

            # ---------- attention ----------
            def attention_block(pair, tcj, tail_hook=None):
                """Pipelined block: sc(si+2) ahead, filler between sc and PV."""
                nk = 4 * tcj + 4
                h1, h2 = 2 * pair, 2 * pair + 1
                qt = QT[pair][tcj]
                pv1 = pvpool.tile([128, 512], F32, tag="pv")
                pv2 = pvpool.tile([128, 512], F32, tag="pv")
                ets = [None] * nk
                widths = [None] * nk

                def emit_scores(si):
                    kt = KT[pair][si // 4]
                    koff = 128 * (si % 4)
                    k = si - 4 * tcj
                    f0 = max(0, 128 * k)
                    w = 512 - f0
                    widths[si] = w
                    ps = pspool.tile([128, 1024], F32, tag="ps")
                    psv = ps.rearrange("p (g f) -> p g f", g=2)
                    diag = k >= 0
                    nc.tensor.matmul(
                        psv[:, 0, f0:512],
                        lhsT=kt[0:64, koff : koff + 128],
                        rhs=qt[0:64, f0:512],
                        start=True,
                        stop=not diag,
                        tile_position=(0, 0),
                    )
                    nc.tensor.matmul(
                        psv[:, 1, f0:512],
                        lhsT=kt[64:128, koff : koff + 128],
                        rhs=qt[64:128, f0:512],
                        start=True,
                        stop=not diag,
                        tile_position=(64, 0),
                    )
                    if diag:
                        # causal triangle: add MASK_BIAS above the diagonal
                        for g in range(2):
                            nc.tensor.matmul(
                                psv[:, g, f0 : f0 + 128],
                                lhsT=ident,
                                rhs=btri,
                                start=False,
                                stop=True,
                            )
                    et = epool.tile([128, 2, 512], F16, tag="e")
                    nc.scalar.activation(
                        out=et[:, :, f0:512],
                        in_=psv[:, :, f0:512],
                        func=mybir.ActivationFunctionType.Exp,
                        scale=SCALE,
                    )
                    ets[si] = et

                def emit_pv(si):
                    f0 = 512 - widths[si]
                    et = ets[si]
                    nc.tensor.matmul(
                        pv1[0:65, f0:512],
                        lhsT=V[si][:, h1, :],
                        rhs=et[:, 0, f0:512],
                        start=(si == 0),
                        stop=(si == nk - 1),
                    )
                    nc.tensor.matmul(
                        pv2[0:65, f0:512],
                        lhsT=V[si][:, h2, :],
                        rhs=et[:, 1, f0:512],
                        start=(si == 0),
                        stop=(si == nk - 1),
                    )

                emit_scores(0)
                emit_scores(1)
                for si in range(nk):
                    if si + 2 < nk:
                        emit_scores(si + 2)
                    # filler to cover exp(si) latency: ACT cost ~ 1.67*w + 140,
                    # PE cost this step ~ (w + 2*w)/2.4 -> deficit
                    w = widths[si]
                    deficit = int(1.67 * w + 140) - int(3 * w / 2.4)
                    take_filler(deficit)
                    if si + 2 >= nk and tail_hook is not None:
                        tail_hook(si)
                    emit_pv(si)

                # ---- normalization: OCT = O / Z ----
                rz = rzpool.tile([1, 1024], F32, tag="rz")
                nc.vector.tensor_copy(rz[:, 0:512], pv1[64:65, :])
                nc.vector.tensor_copy(rz[:, 512:1024], pv2[64:65, :])
                nc.vector.reciprocal_approx_fast(out=rz, in_=rz)
                rzb = rzpool.tile([64, 1024], F32, tag="rzb")
                nc.gpsimd.partition_broadcast(rzb, rz)
                oct_t = octpool.tile([128, 512], F16, tag=f"oct{pair}_{tcj}")
                OCT[pair][tcj] = oct_t
                nc.vector.tensor_mul(oct_t[0:64, :], pv1[0:64, :], rzb[:, 0:512])
                nc.vector.tensor_mul(oct_t[64:128, :], pv2[0:64, :], rzb[:, 512:1024])

            # ---------- prologue: V(0..3), QK(tj=0) ----------
            for s in range(4):
                for _, emit in v_chunk_items(s):
                    emit()
            for n in range(8):
                for _, emit in qk_chunk_items(0, n):
                    emit()

            # ---------- rounds ----------
            # round 0 filler: V(4..7) + qk(1); round 1: V(8..11)+qk(2)+proj(0..3)
            # round 2: V(12..15)+qk(3); round 3: proj(4..11)
            for tcj in range(TJ):
                if tcj < 3:
                    for s in range(4 * (tcj + 1), 4 * (tcj + 2)):
                        filler.extend(v_chunk_items(s))
                    for n in range(8):
                        filler.extend(qk_chunk_items(tcj + 1, n))
                    if tcj == 1:
                        for t2 in range(0, 4):
                            for cj in range(2):
                                filler.extend(proj_chunk_items(t2, cj))
                else:
                    for t2 in range(4, 12):
                        for cj in range(2):
                            filler.extend(proj_chunk_items(t2, cj))

                for pair in range(PAIRS):
                    attention_block(pair, tcj)
                drain_filler()

            # ---------- tail: proj of OCT[.][3] (t2 12..15) ----------
            for t2 in range(12, 16):
                for cj in range(2):
                    for _, emit in proj_chunk_items(t2, cj, evict_on_act=True):
                        emit()

    nc.finalize()
    return nc


def _get_nc():
    if "nc" not in _CACHE:
        _CACHE["nc"] = _build()
    return _CACHE["nc"]


def kernel(x, W_qkv, b_qkv, W_proj, b_proj):
    from concourse.bass_utils import run_bass_kernel_spmd

    x = np.asarray(x, dtype=np.float32)
    W_qkv = np.asarray(W_qkv, dtype=np.float32)
    b_qkv = np.asarray(b_qkv, dtype=np.float32)
    W_proj = np.asarray(W_proj, dtype=np.float32)
    b_proj = np.asarray(b_proj, dtype=np.float32)

    in_maps = []
    for core in range(N_CORES):
        b = core // 2
        h2 = core % 2
        o = 512 * h2
        xt = np.ascontiguousarray(x[b].T).astype(np.float16)
        wq = W_qkv[:, o : o + 512]
        wk = W_qkv[:, 1024 + o : 1024 + o + 512]
        wqk = np.ascontiguousarray(np.concatenate([wq, wk], axis=1)).astype(np.float16)
        wv = np.ascontiguousarray(W_qkv[:, 2048 + o : 2048 + o + 512]).astype(np.float16)
        wp = np.ascontiguousarray(W_proj[o : o + 512, :]).astype(np.float16)
        bq = b_qkv[o : o + 512]
        bk = b_qkv[1024 + o : 1024 + o + 512]
        bqk = np.ascontiguousarray(np.concatenate([bq, bk]).reshape(8, 128).T)
        bv = np.ascontiguousarray(
            b_qkv[2048 + o : 2048 + o + 512].reshape(1, 512)
        ).astype(np.float16)
        in_maps.append(
            {"xt": xt, "wqk": wqk, "wv": wv, "wp": wp, "bqk": bqk, "bv": bv}
        )

    nc = _get_nc()
    kwargs = {}
    if os.environ.get("BASS_KERNEL_TRACE"):
        kwargs["trace"] = True
    res = run_bass_kernel_spmd(nc, in_maps, core_ids=list(range(N_CORES)), **kwargs)
    _CACHE["last_results"] = res

    out = np.empty((B, T, N_EMBD), dtype=np.float32)
    for b in range(B):
        out[b] = (
            res.results[2 * b]["out_p"].astype(np.float32)
            + res.results[2 * b + 1]["out_p"].astype(np.float32)
            + b_proj[None, :]
        )
    return out


# revision 8
# speedup vs baseline: 1.1420x; 1.1420x over previous
"""Causal self-attention Bass kernel for TRN2, 8 NeuronCores.

Sharding: data-parallel over batch (B=4) x tensor-parallel over head halves
(2 groups of 8 heads) = 8 shards, Megatron-style. Each core computes its
batch's qkv projection for its 8 heads, causal attention, and a partial
output projection (its heads' rows of W_proj). The host sums the two
partials per batch and adds b_proj.

All matmul operands are fp16 (full-rate 1 cycle/row on the PE, fp32 PSUM
accumulation). Output partials are stored fp16 (halves DMA; host sums in
fp32).

Schedule (v2): the per-si chain is scores -> exp(ACT) -> PV, with ACT and
PE in near-lockstep. The emission software-pipelines each attention block
(scores si+2 ahead of PV si) and injects filler matmuls (V/QK projections
for later rounds, output-projection chunks for earlier rounds) between the
scores and PV of each si so the PE never waits on the exp. The causal mask
is applied as a -30000 bias matmul into PSUM on diagonal chunks before the
exp (fp16 underflow -> exact zeros), keeping DVE off the critical chain.

Layouts per core:
  xt   = x[b].T (fp16)                 (C=1024, T=2048), 32 [128,512] tiles
  wqk  = [Wq_half | Wk_half] (fp16)    (1024, 1024)
  wv   = Wv_half (fp16)                (1024, 512)
  wp   = W_proj[512*h2:+512, :] (fp16) (512, 1024)
  QT/KT tiles [128, 512] fp16: partitions = d + 64*(h%2) for head pair h//2
  V tiles [128, 8, 65] fp16: per s-chunk, 8 heads x (64 V cols + ones col)
  scores^T [s,t] 2-head row-packed (concurrent row-tiled MMs), diag cols
  clipped + triangle bias-masked -> ACT exp -> PV matmul M=65 -> O^T[d,t]
  + Z row in PSUM -> recip_approx_fast + gpsimd partition_broadcast ->
  normalized OCT (SBUF) -> proj: out[t, c] = sum_hd OCT[hd, t] * wp[hd, c]
"""

import math
import os
from collections import deque

import numpy as np

import concourse.bass as bass
import concourse.mybir as mybir
from concourse import bacc
from concourse.tile import TileContext

F32 = mybir.dt.float32
F16 = mybir.dt.float16

N_EMBD = 1024
N_HEAD = 16
D = 64
B = 4
T = 2048
N_CORES = 8
PAIRS = 4          # head pairs per core (8 heads)
TJ = T // 512      # 512-wide t super-chunks
SJ = T // 128      # 128-wide s chunks
SCALE = 1.0 / math.sqrt(D)
MASK_BIAS = -30000.0

_CACHE = {}


def _build():
    nc = bacc.Bacc()

    xt_d = nc.declare_dram_parameter("xt", [N_EMBD, T], F16, isOutput=False)
    wqk_d = nc.declare_dram_parameter("wqk", [N_EMBD, 1024], F16, isOutput=False)
    wv_d = nc.declare_dram_parameter("wv", [N_EMBD, 512], F16, isOutput=False)
    wp_d = nc.declare_dram_parameter("wp", [512, N_EMBD], F16, isOutput=False)
    bqk_d = nc.declare_dram_parameter("bqk", [128, 8], F32, isOutput=False)
    bv_d = nc.declare_dram_parameter("bv", [1, 512], F16, isOutput=False)
    out_d = nc.declare_dram_parameter("out_p", [T, N_EMBD], F16, isOutput=True)

    with TileContext(nc) as tc:
        with (
            tc.tile_pool(name="const", bufs=1) as cpool,
            tc.tile_pool(name="w", bufs=1) as wpool,
            tc.tile_pool(name="xt", bufs=1) as xpool,
            tc.tile_pool(name="qkt", bufs=1) as qkpool,
            tc.tile_pool(name="v", bufs=1) as vpool,
            tc.tile_pool(name="e", bufs=4) as epool,
            tc.tile_pool(name="octp", bufs=1) as octpool,
            tc.tile_pool(name="rz", bufs=2) as rzpool,
            tc.tile_pool(name="spv", bufs=2) as spvpool,
            tc.tile_pool(name="outp", bufs=4) as opool,
            tc.tile_pool(name="ps", bufs=2, space="PSUM") as pspool,
            tc.tile_pool(name="pv", bufs=2, space="PSUM") as pvpool,
            tc.tile_pool(name="fl", bufs=2, space="PSUM") as flpool,
        ):
            # ---- constants ----
            ones_f = cpool.tile([1, 128], F32, tag="ones_f")
            nc.vector.memset(ones_f, 1.0)
            ones_r = cpool.tile([1, 128], F16, tag="ones_r")
            nc.vector.tensor_copy(ones_r, ones_f)
            ones8 = cpool.tile([128, 8], F32, tag="ones8")
            nc.vector.memset(ones8, 1.0)
            bqk_t = cpool.tile([128, 8], F32, tag="bqk")
            nc.scalar.dma_start(out=bqk_t, in_=bqk_d[:, :])
            bv_t = cpool.tile([1, 512], F16, tag="bv")
            nc.scalar.dma_start(out=bv_t, in_=bv_d[:, :])

            # identity [128,128] fp16 (bias-matmul lhsT)
            ident = cpool.tile([128, 128], F16, tag="ident")
            nc.vector.memset(ident, 1.0)
            nc.gpsimd.affine_select(
                out=ident, in_=ident, compare_op=mybir.AluOpType.is_ge, fill=0.0,
                base=0, pattern=[[1, 128]], channel_multiplier=-1,
            )
            nc.gpsimd.affine_select(
                out=ident, in_=ident, compare_op=mybir.AluOpType.is_ge, fill=0.0,
                base=0, pattern=[[-1, 128]], channel_multiplier=1,
            )
            # btri[p, j] = 0 where j >= p else MASK_BIAS  (causal triangle)
            btri = cpool.tile([128, 128], F16, tag="btri")
            nc.vector.memset(btri, 0.0)
            nc.gpsimd.affine_select(
                out=btri, in_=btri, compare_op=mybir.AluOpType.is_ge,
                fill=MASK_BIAS, base=0, pattern=[[1, 128]], channel_multiplier=-1,
            )

            # ---- weight/x DMAs: wv+x on sync queue, wqk/wp on vector queue ----
            wqk = []
            wv = []
            wp = []
            XT = [[None] * TJ for _ in range(8)]  # [c][tj] -> [128, 512]
            for c in range(8):
                t = wpool.tile([128, 512], F16, tag=f"wv{c}")
                nc.sync.dma_start(out=t, in_=wv_d[128 * c : 128 * c + 128, :])
                wv.append(t)
                tx = xpool.tile([128, 512], F16, tag=f"x{c}_0")
                nc.sync.dma_start(out=tx, in_=xt_d[128 * c : 128 * c + 128, 0:512])
                XT[c][0] = tx
            for c in range(8):
                t = wpool.tile([128, 1024], F16, tag=f"wqk{c}")
                nc.scalar.dma_start(out=t, in_=wqk_d[128 * c : 128 * c + 128, :])
                wqk.append(t)
            for tj in range(1, TJ):
                for c in range(8):
                    tx = xpool.tile([128, 512], F16, tag=f"x{c}_{tj}")
                    nc.sync.dma_start(
                        out=tx,
                        in_=xt_d[128 * c : 128 * c + 128, 512 * tj : 512 * tj + 512],
                    )
                    XT[c][tj] = tx
            for p in range(PAIRS):
                t = wpool.tile([128, 1024], F16, tag=f"wp{p}")
                nc.scalar.dma_start(out=t, in_=wp_d[128 * p : 128 * p + 128, :])
                wp.append(t)

            # bvb = b_v broadcast to [128, 512] via K=1 matmul
            ps_bvb = flpool.tile([128, 512], F32, tag="f")
            nc.tensor.matmul(
                ps_bvb, lhsT=ones_r[0:1, :], rhs=bv_t, start=True, stop=True
            )
            bvb = cpool.tile([128, 512], F32, tag="bvb")
            nc.vector.tensor_copy(bvb, ps_bvb)

            QT = [[None] * TJ for _ in range(PAIRS)]
            KT = [[None] * TJ for _ in range(PAIRS)]
            V = [None] * SJ
            OCT = [[None] * TJ for _ in range(PAIRS)]

            # ---------- filler machinery ----------
            # Each filler item: (cost_ns, closure). Closures emit ONE matmul,
            # plus evictions when they complete a chunk.
            filler = deque()

            def v_chunk_items(s_idx):
                tj, sj = s_idx // 4, s_idx % 4
                state = {}

                def mk(c):
                    def emit():
                        if c == 0:
                            state["pv"] = flpool.tile([128, 512], F32, tag="f", name="flv")
                        nc.tensor.matmul(
                            state["pv"],
                            lhsT=XT[c][tj][:, 128 * sj : 128 * sj + 128],
                            rhs=wv[c],
                            start=(c == 0),
                            stop=(c == 7),
                        )
                        if c == 7:
                            vt = vpool.tile([128, 8, 65], F16, tag=f"v{s_idx}")
                            nc.vector.tensor_add(
                                vt[:, :, 0:64],
                                state["pv"].rearrange("p (h d) -> p h d", h=8),
                                bvb.rearrange("p (h d) -> p h d", h=8),
                            )
                            nc.vector.tensor_copy(
                                vt[:, :, 64:65],
                                ones8.rearrange("p (h o) -> p h o", h=8),
                            )
                            V[s_idx] = vt
                    return emit

                return [(213, mk(c)) for c in range(8)]

            def qk_chunk_items(tj, n):
                state = {}

                def mk(c):
                    def emit():
                        if c == 0:
                            state["ps"] = flpool.tile([128, 512], F32, tag="f", name="flqk")
                        nc.tensor.matmul(
                            state["ps"],
                            lhsT=wqk[c][:, 128 * n : 128 * n + 128],
                            rhs=XT[c][tj],
                            start=(c == 0),
                            stop=(c == 7),
                        )
                        if c == 7:
                            dst = qkpool.tile([128, 512], F16, tag=f"qk{n}_{tj}")
                            nc.vector.tensor_scalar_add(
                                dst, state["ps"], bqk_t[:, n : n + 1]
                            )
                            if n < 4:
                                QT[n][tj] = dst
                            else:
                                KT[n - 4][tj] = dst
                    return emit

                return [(213, mk(c)) for c in range(8)]

            def proj_chunk_items(t2, cj, evict_on_act=False):
                tcj, k = t2 // 4, t2 % 4
                state = {}

                def mk(pair):
                    def emit():
                        if pair == 0:
                            state["po"] = flpool.tile([128, 512], F32, tag="f", name="flpo")
                        nc.tensor.matmul(
                            state["po"],
                            lhsT=OCT[pair][tcj][:, 128 * k : 128 * k + 128],
                            rhs=wp[pair][:, 512 * cj : 512 * cj + 512],
                            start=(pair == 0),
                            stop=(pair == 3),
                        )
                        if pair == 3:
                            ot = opool.tile([128, 512], F16, tag="out")
                            if evict_on_act:
                                nc.scalar.copy(ot, state["po"])
                            else:
                                nc.vector.tensor_copy(ot, state["po"])
                            nc.gpsimd.dma_start(
                                out=out_d[
                                    128 * t2 : 128 * t2 + 128,
                                    512 * cj : 512 * cj + 512,
                                ],
                                in_=ot,
                            )
                    return emit

                return [(213, mk(pair)) for pair in range(PAIRS)]

            def drain_filler(n=None):
                cnt = len(filler) if n is None else min(n, len(filler))
                for _ in range(cnt):
                    _, emit = filler.popleft()
                    emit()

            def take_filler(budget_ns):
                """Emit filler until ~budget_ns of PE work injected."""
                spent = 0
                while filler and spent < budget_ns:
                    cost, emit = filler.popleft()
                    emit()
                    spent += cost
                return spent

            # ---------- attention ----------
            def attention_block(pair, tcj, mid_hook=None):
                """Pipelined block: sc(si+2) ahead, filler between sc and PV."""
                nk = 4 * tcj + 4
                h1, h2 = 2 * pair, 2 * pair + 1
                qt = QT[pair][tcj]
                pv1 = pvpool.tile([128, 512], F32, tag="pv")
                pv2 = pvpool.tile([128, 512], F32, tag="pv")
                ets = [None] * nk
                widths = [None] * nk

                def emit_scores(si):
                    kt = KT[pair][si // 4]
                    koff = 128 * (si % 4)
                    k = si - 4 * tcj
                    f0 = max(0, 128 * k)
                    w = 512 - f0
                    widths[si] = w
                    ps = pspool.tile([128, 1024], F32, tag="ps")
                    psv = ps.rearrange("p (g f) -> p g f", g=2)
                    diag = k >= 0
                    nc.tensor.matmul(
                        psv[:, 0, f0:512],
                        lhsT=kt[0:64, koff : koff + 128],
                        rhs=qt[0:64, f0:512],
                        start=True,
                        stop=not diag,
                        tile_position=(0, 0),
                    )
                    nc.tensor.matmul(
                        psv[:, 1, f0:512],
                        lhsT=kt[64:128, koff : koff + 128],
                        rhs=qt[64:128, f0:512],
                        start=True,
                        stop=not diag,
                        tile_position=(64, 0),
                    )
                    if diag:
                        # causal triangle: add MASK_BIAS above the diagonal
                        for g in range(2):
                            nc.tensor.matmul(
                                psv[:, g, f0 : f0 + 128],
                                lhsT=ident,
                                rhs=btri,
                                start=False,
                                stop=True,
                            )
                    et = epool.tile([128, 2, 512], F16, tag="e")
                    nc.scalar.activation(
                        out=et[:, :, f0:512],
                        in_=psv[:, :, f0:512],
                        func=mybir.ActivationFunctionType.Exp,
                        scale=SCALE,
                    )
                    ets[si] = et

                def emit_pv(si):
                    f0 = 512 - widths[si]
                    et = ets[si]
                    nc.tensor.matmul(
                        pv1[0:65, f0:512],
                        lhsT=V[si][:, h1, :],
                        rhs=et[:, 0, f0:512],
                        start=(si == 0),
                        stop=(si == nk - 1),
                    )
                    nc.tensor.matmul(
                        pv2[0:65, f0:512],
                        lhsT=V[si][:, h2, :],
                        rhs=et[:, 1, f0:512],
                        start=(si == 0),
                        stop=(si == nk - 1),
                    )

                emit_scores(0)
                emit_scores(1)
                for si in range(nk):
                    if si + 2 < nk:
                        emit_scores(si + 2)
                    # filler to cover exp(si) latency: ACT cost ~ 1.67*w + 140,
                    # PE cost this step ~ (w + 2*w)/2.4 -> deficit
                    w = widths[si]
                    deficit = int(1.67 * w + 140) - int(3 * w / 2.4)
                    take_filler(deficit)
                    if si == nk - 2 and mid_hook is not None:
                        mid_hook()
                    emit_pv(si)

                # ---- normalization: OCT = O / Z (staged via SBUF to free
                # the PV psum banks quickly for the next block) ----
                spv = spvpool.tile([128, 1024], F32, tag="spv")
                nc.vector.tensor_copy(spv[0:65, 0:512], pv1[0:65, :])
                nc.vector.tensor_copy(spv[0:65, 512:1024], pv2[0:65, :])
                rz = rzpool.tile([1, 1024], F32, tag="rz")
                nc.vector.tensor_copy(rz, spv[64:65, :])
                nc.vector.reciprocal_approx_fast(out=rz, in_=rz)
                rzb = rzpool.tile([64, 1024], F32, tag="rzb")
                nc.gpsimd.partition_broadcast(rzb, rz)
                oct_t = octpool.tile([128, 512], F16, tag=f"oct{pair}_{tcj}")
                OCT[pair][tcj] = oct_t
                nc.vector.tensor_mul(
                    oct_t[0:64, :], spv[0:64, 0:512], rzb[:, 0:512]
                )
                nc.vector.tensor_mul(
                    oct_t[64:128, :], spv[0:64, 512:1024], rzb[:, 512:1024]
                )

            # ---------- prologue: V(0..3), QK(tj=0) for pair 0 ----------
            for s in range(4):
                for _, emit in v_chunk_items(s):
                    emit()
            for n in (0, 4):
                for _, emit in qk_chunk_items(0, n):
                    emit()
            # pair p's block in round 0 needs qk(0, {p, 4+p}): feed them first
            for p in (1, 2, 3):
                filler.extend(qk_chunk_items(0, p))
                filler.extend(qk_chunk_items(0, 4 + p))

            # ---------- rounds ----------
            # round 0 filler: V(4..7) + qk(1); round 1: V(8..11)+qk(2)+proj(0..3)
            # round 2: V(12..15)+qk(3); round 3: proj(4..11)
            tail_state = {}

            def tail_mid_hook():
                # during the last block, pre-accumulate pairs 0..2 of the
                # t2=12,13 proj chunks into the (now free) scores psum tiles
                drain_filler()
                for t2 in (12, 13):
                    po = pspool.tile([128, 1024], F32, tag="ps", name="tailpo")
                    tail_state[t2] = po
                    pov = po.rearrange("p (g f) -> p g f", g=2)
                    for cj in range(2):
                        for p3 in range(3):
                            nc.tensor.matmul(
                                pov[:, cj, :],
                                lhsT=OCT[p3][3][:, 128 * (t2 % 4) : 128 * (t2 % 4) + 128],
                                rhs=wp[p3][:, 512 * cj : 512 * cj + 512],
                                start=(p3 == 0),
                                stop=False,
                            )

            tail_state["hook"] = tail_mid_hook

            for tcj in range(TJ):
                if tcj < 3:
                    for s in range(4 * (tcj + 1), 4 * (tcj + 2)):
                        filler.extend(v_chunk_items(s))
                    for n in range(8):
                        filler.extend(qk_chunk_items(tcj + 1, n))
                    if tcj == 1:
                        for t2 in range(0, 4):
                            for cj in range(2):
                                filler.extend(proj_chunk_items(t2, cj))
                else:
                    for t2 in range(4, 12):
                        for cj in range(2):
                            filler.extend(proj_chunk_items(t2, cj))

                r0_total = len(filler) if tcj == 0 else 0
                for pair in range(PAIRS):
                    if tcj == 0 and pair > 0:
                        # qk(0) chunks for this pair must be emitted before its
                        # block reads QT/KT: first 16*pair items of the deque
                        need = 16 * pair - (r0_total - len(filler))
                        if need > 0:
                            drain_filler(need)
                    hook = tail_state["hook"] if (tcj == 3 and pair == 3) else None
                    attention_block(pair, tcj, mid_hook=hook)
                drain_filler()

            # ---------- tail: pair-3 matmuls of t2 12,13 then t2 14,15 ----------
            for t2 in (12, 13):
                po = tail_state[t2]
                pov = po.rearrange("p (g f) -> p g f", g=2)
                for cj in range(2):
                    nc.tensor.matmul(
                        pov[:, cj, :],
                        lhsT=OCT[3][3][:, 128 * (t2 % 4) : 128 * (t2 % 4) + 128],
                        rhs=wp[3][:, 512 * cj : 512 * cj + 512],
                        start=False,
                        stop=True,
                    )
                    ot = opool.tile([128, 512], F16, tag="out", name="ott")
                    nc.scalar.copy(ot, pov[:, cj, :])
                    nc.gpsimd.dma_start(
                        out=out_d[128 * t2 : 128 * t2 + 128, 512 * cj : 512 * cj + 512],
                        in_=ot,
                    )
            for t2 in (14, 15):
                for cj in range(2):
                    for _, emit in proj_chunk_items(t2, cj, evict_on_act=True):
                        emit()

    nc.finalize()
    return nc


def _get_nc():
    if "nc" not in _CACHE:
        _CACHE["nc"] = _build()
    return _CACHE["nc"]


def kernel(x, W_qkv, b_qkv, W_proj, b_proj):
    from concourse.bass_utils import run_bass_kernel_spmd

    x = np.asarray(x, dtype=np.float32)
    W_qkv = np.asarray(W_qkv, dtype=np.float32)
    b_qkv = np.asarray(b_qkv, dtype=np.float32)
    W_proj = np.asarray(W_proj, dtype=np.float32)
    b_proj = np.asarray(b_proj, dtype=np.float32)

    in_maps = []
    for core in range(N_CORES):
        b = core // 2
        h2 = core % 2
        o = 512 * h2
        xt = np.ascontiguousarray(x[b].T).astype(np.float16)
        wq = W_qkv[:, o : o + 512]
        wk = W_qkv[:, 1024 + o : 1024 + o + 512]
        wqk = np.ascontiguousarray(np.concatenate([wq, wk], axis=1)).astype(np.float16)
        wv = np.ascontiguousarray(W_qkv[:, 2048 + o : 2048 + o + 512]).astype(np.float16)
        wp = np.ascontiguousarray(W_proj[o : o + 512, :]).astype(np.float16)
        bq = b_qkv[o : o + 512]
        bk = b_qkv[1024 + o : 1024 + o + 512]
        bqk = np.ascontiguousarray(np.concatenate([bq, bk]).reshape(8, 128).T)
        bv = np.ascontiguousarray(
            b_qkv[2048 + o : 2048 + o + 512].reshape(1, 512)
        ).astype(np.float16)
        in_maps.append(
            {"xt": xt, "wqk": wqk, "wv": wv, "wp": wp, "bqk": bqk, "bv": bv}
        )

    nc = _get_nc()
    kwargs = {}
    if os.environ.get("BASS_KERNEL_TRACE"):
        kwargs["trace"] = True
    res = run_bass_kernel_spmd(nc, in_maps, core_ids=list(range(N_CORES)), **kwargs)
    _CACHE["last_results"] = res

    out = np.empty((B, T, N_EMBD), dtype=np.float32)
    for b in range(B):
        out[b] = (
            res.results[2 * b]["out_p"].astype(np.float32)
            + res.results[2 * b + 1]["out_p"].astype(np.float32)
            + b_proj[None, :]
        )
    return out


# revision 9
# speedup vs baseline: 1.1430x; 1.0009x over previous
"""Causal self-attention Bass kernel for TRN2, 8 NeuronCores.

Sharding: data-parallel over batch (B=4) x tensor-parallel over head halves
(2 groups of 8 heads) = 8 shards, Megatron-style. Each core computes its
batch's qkv projection for its 8 heads, causal attention, and a partial
output projection (its heads' rows of W_proj). The host sums the two
partials per batch and adds b_proj.

All matmul operands are fp16 (full-rate 1 cycle/row on the PE, fp32 PSUM
accumulation). Output partials are stored fp16 (halves DMA; host sums in
fp32).

Schedule (v2): the per-si chain is scores -> exp(ACT) -> PV, with ACT and
PE in near-lockstep. The emission software-pipelines each attention block
(scores si+2 ahead of PV si) and injects filler matmuls (V/QK projections
for later rounds, output-projection chunks for earlier rounds) between the
scores and PV of each si so the PE never waits on the exp. The causal mask
is applied as a -30000 bias matmul into PSUM on diagonal chunks before the
exp (fp16 underflow -> exact zeros), keeping DVE off the critical chain.

Layouts per core:
  xt   = x[b].T (fp16)                 (C=1024, T=2048), 32 [128,512] tiles
  wqk  = [Wq_half | Wk_half] (fp16)    (1024, 1024)
  wv   = Wv_half (fp16)                (1024, 512)
  wp   = W_proj[512*h2:+512, :] (fp16) (512, 1024)
  QT/KT tiles [128, 512] fp16: partitions = d + 64*(h%2) for head pair h//2
  V tiles [128, 8, 65] fp16: per s-chunk, 8 heads x (64 V cols + ones col)
  scores^T [s,t] 2-head row-packed (concurrent row-tiled MMs), diag cols
  clipped + triangle bias-masked -> ACT exp -> PV matmul M=65 -> O^T[d,t]
  + Z row in PSUM -> recip_approx_fast + gpsimd partition_broadcast ->
  normalized OCT (SBUF) -> proj: out[t, c] = sum_hd OCT[hd, t] * wp[hd, c]
"""

import math
import os
from collections import deque

import numpy as np

import concourse.bass as bass
import concourse.mybir as mybir
from concourse import bacc
from concourse.tile import TileContext

F32 = mybir.dt.float32
F16 = mybir.dt.float16

N_EMBD = 1024
N_HEAD = 16
D = 64
B = 4
T = 2048
N_CORES = 8
PAIRS = 4          # head pairs per core (8 heads)
TJ = T // 512      # 512-wide t super-chunks
SJ = T // 128      # 128-wide s chunks
SCALE = 1.0 / math.sqrt(D)
MASK_BIAS = -30000.0

_CACHE = {}


def _build():
    nc = bacc.Bacc()

    xt_d = nc.declare_dram_parameter("xt", [N_EMBD, T], F16, isOutput=False)
    wqk_d = nc.declare_dram_parameter("wqk", [N_EMBD, 1024], F16, isOutput=False)
    wv_d = nc.declare_dram_parameter("wv", [N_EMBD, 512], F16, isOutput=False)
    wp_d = nc.declare_dram_parameter("wp", [512, N_EMBD], F16, isOutput=False)
    bqk_d = nc.declare_dram_parameter("bqk", [128, 8], F32, isOutput=False)
    bv_d = nc.declare_dram_parameter("bv", [1, 512], F16, isOutput=False)
    out_d = nc.declare_dram_parameter("out_p", [T, N_EMBD], F16, isOutput=True)

    with TileContext(nc) as tc:
        with (
            tc.tile_pool(name="const", bufs=1) as cpool,
            tc.tile_pool(name="w", bufs=1) as wpool,
            tc.tile_pool(name="xt", bufs=1) as xpool,
            tc.tile_pool(name="qkt", bufs=1) as qkpool,
            tc.tile_pool(name="v", bufs=1) as vpool,
            tc.tile_pool(name="e", bufs=4) as epool,
            tc.tile_pool(name="octp", bufs=1) as octpool,
            tc.tile_pool(name="rz", bufs=2) as rzpool,
            tc.tile_pool(name="spv", bufs=2) as spvpool,
            tc.tile_pool(name="outp", bufs=4) as opool,
            tc.tile_pool(name="ps", bufs=2, space="PSUM") as pspool,
            tc.tile_pool(name="pv", bufs=2, space="PSUM") as pvpool,
            tc.tile_pool(name="fl", bufs=2, space="PSUM") as flpool,
        ):
            # ---- constants ----
            ones_f = cpool.tile([1, 128], F32, tag="ones_f")
            nc.vector.memset(ones_f, 1.0)
            ones_r = cpool.tile([1, 128], F16, tag="ones_r")
            nc.vector.tensor_copy(ones_r, ones_f)
            ones8 = cpool.tile([128, 8], F32, tag="ones8")
            nc.vector.memset(ones8, 1.0)
            bqk_t = cpool.tile([128, 8], F32, tag="bqk")
            nc.scalar.dma_start(out=bqk_t, in_=bqk_d[:, :])
            bv_t = cpool.tile([1, 512], F16, tag="bv")
            nc.scalar.dma_start(out=bv_t, in_=bv_d[:, :])

            # identity [128,128] fp16 (bias-matmul lhsT)
            ident = cpool.tile([128, 128], F16, tag="ident")
            nc.vector.memset(ident, 1.0)
            nc.gpsimd.affine_select(
                out=ident, in_=ident, compare_op=mybir.AluOpType.is_ge, fill=0.0,
                base=0, pattern=[[1, 128]], channel_multiplier=-1,
            )
            nc.gpsimd.affine_select(
                out=ident, in_=ident, compare_op=mybir.AluOpType.is_ge, fill=0.0,
                base=0, pattern=[[-1, 128]], channel_multiplier=1,
            )
            # btri[p, j] = 0 where j >= p else MASK_BIAS  (causal triangle)
            btri = cpool.tile([128, 128], F16, tag="btri")
            nc.vector.memset(btri, 0.0)
            nc.gpsimd.affine_select(
                out=btri, in_=btri, compare_op=mybir.AluOpType.is_ge,
                fill=MASK_BIAS, base=0, pattern=[[1, 128]], channel_multiplier=-1,
            )

            # ---- weight/x DMAs: wv+x on sync queue, wqk/wp on vector queue ----
            wqk = []
            wv = []
            wp = []
            XT = [[None] * TJ for _ in range(8)]  # [c][tj] -> [128, 512]
            for c in range(8):
                t = wpool.tile([128, 512], F16, tag=f"wv{c}")
                eng = nc.scalar if c < 4 else nc.sync
                eng.dma_start(out=t, in_=wv_d[128 * c : 128 * c + 128, :])
                wv.append(t)
                tx = xpool.tile([128, 512], F16, tag=f"x{c}_0")
                nc.sync.dma_start(out=tx, in_=xt_d[128 * c : 128 * c + 128, 0:512])
                XT[c][0] = tx
            for c in range(8):
                t = wpool.tile([128, 1024], F16, tag=f"wqk{c}")
                nc.scalar.dma_start(out=t, in_=wqk_d[128 * c : 128 * c + 128, :])
                wqk.append(t)
            for tj in range(1, TJ):
                for c in range(8):
                    tx = xpool.tile([128, 512], F16, tag=f"x{c}_{tj}")
                    nc.sync.dma_start(
                        out=tx,
                        in_=xt_d[128 * c : 128 * c + 128, 512 * tj : 512 * tj + 512],
                    )
                    XT[c][tj] = tx
            for p in range(PAIRS):
                t = wpool.tile([128, 1024], F16, tag=f"wp{p}")
                nc.scalar.dma_start(out=t, in_=wp_d[128 * p : 128 * p + 128, :])
                wp.append(t)

            # bvb = b_v broadcast to [128, 512] via K=1 matmul
            ps_bvb = flpool.tile([128, 512], F32, tag="f")
            nc.tensor.matmul(
                ps_bvb, lhsT=ones_r[0:1, :], rhs=bv_t, start=True, stop=True
            )
            bvb = cpool.tile([128, 512], F32, tag="bvb")
            nc.vector.tensor_copy(bvb, ps_bvb)

            QT = [[None] * TJ for _ in range(PAIRS)]
            KT = [[None] * TJ for _ in range(PAIRS)]
            V = [None] * SJ
            OCT = [[None] * TJ for _ in range(PAIRS)]

            def interleave(a, b):
                out = []
                for x, y in zip(a, b):
                    out.append(x)
                    out.append(y)
                return out

            # ---------- filler machinery ----------
            # Each filler item: (cost_ns, closure). Closures emit ONE matmul,
            # plus evictions when they complete a chunk.
            filler = deque()

            def v_chunk_items(s_idx):
                tj, sj = s_idx // 4, s_idx % 4
                state = {}

                def mk(c):
                    def emit():
                        if c == 0:
                            state["pv"] = flpool.tile([128, 512], F32, tag="f", name="flv")
                        nc.tensor.matmul(
                            state["pv"],
                            lhsT=XT[c][tj][:, 128 * sj : 128 * sj + 128],
                            rhs=wv[c],
                            start=(c == 0),
                            stop=(c == 7),
                        )
                        if c == 7:
                            vt = vpool.tile([128, 8, 65], F16, tag=f"v{s_idx}")
                            nc.vector.tensor_add(
                                vt[:, :, 0:64],
                                state["pv"].rearrange("p (h d) -> p h d", h=8),
                                bvb.rearrange("p (h d) -> p h d", h=8),
                            )
                            nc.vector.tensor_copy(
                                vt[:, :, 64:65],
                                ones8.rearrange("p (h o) -> p h o", h=8),
                            )
                            V[s_idx] = vt
                    return emit

                return [(213, mk(c)) for c in range(8)]

            def qk_chunk_items(tj, n):
                state = {}

                def mk(c):
                    def emit():
                        if c == 0:
                            state["ps"] = flpool.tile([128, 512], F32, tag="f", name="flqk")
                        nc.tensor.matmul(
                            state["ps"],
                            lhsT=wqk[c][:, 128 * n : 128 * n + 128],
                            rhs=XT[c][tj],
                            start=(c == 0),
                            stop=(c == 7),
                        )
                        if c == 7:
                            dst = qkpool.tile([128, 512], F16, tag=f"qk{n}_{tj}")
                            nc.vector.tensor_scalar_add(
                                dst, state["ps"], bqk_t[:, n : n + 1]
                            )
                            if n < 4:
                                QT[n][tj] = dst
                            else:
                                KT[n - 4][tj] = dst
                    return emit

                return [(213, mk(c)) for c in range(8)]

            def proj_chunk_items(t2, cj, evict_on_act=False):
                tcj, k = t2 // 4, t2 % 4
                state = {}

                def mk(pair):
                    def emit():
                        if pair == 0:
                            state["po"] = flpool.tile([128, 512], F32, tag="f", name="flpo")
                        nc.tensor.matmul(
                            state["po"],
                            lhsT=OCT[pair][tcj][:, 128 * k : 128 * k + 128],
                            rhs=wp[pair][:, 512 * cj : 512 * cj + 512],
                            start=(pair == 0),
                            stop=(pair == 3),
                        )
                        if pair == 3:
                            ot = opool.tile([128, 512], F16, tag="out")
                            if evict_on_act:
                                nc.scalar.copy(ot, state["po"])
                            else:
                                nc.vector.tensor_copy(ot, state["po"])
                            nc.gpsimd.dma_start(
                                out=out_d[
                                    128 * t2 : 128 * t2 + 128,
                                    512 * cj : 512 * cj + 512,
                                ],
                                in_=ot,
                            )
                    return emit

                return [(213, mk(pair)) for pair in range(PAIRS)]

            def drain_filler(n=None):
                cnt = len(filler) if n is None else min(n, len(filler))
                for _ in range(cnt):
                    _, emit = filler.popleft()
                    emit()

            def take_filler(budget_ns):
                """Emit filler until ~budget_ns of PE work injected."""
                spent = 0
                while filler and spent < budget_ns:
                    cost, emit = filler.popleft()
                    emit()
                    spent += cost
                return spent

            # ---------- attention ----------
            def attention_block(pair, tcj, mid_hook=None):
                """Pipelined block: sc(si+2) ahead, filler between sc and PV."""
                nk = 4 * tcj + 4
                h1, h2 = 2 * pair, 2 * pair + 1
                qt = QT[pair][tcj]
                pv1 = pvpool.tile([128, 512], F32, tag="pv")
                pv2 = pvpool.tile([128, 512], F32, tag="pv")
                ets = [None] * nk
                widths = [None] * nk

                def emit_scores(si):
                    kt = KT[pair][si // 4]
                    koff = 128 * (si % 4)
                    k = si - 4 * tcj
                    f0 = max(0, 128 * k)
                    w = 512 - f0
                    widths[si] = w
                    ps = pspool.tile([128, 1024], F32, tag="ps")
                    psv = ps.rearrange("p (g f) -> p g f", g=2)
                    diag = k >= 0
                    nc.tensor.matmul(
                        psv[:, 0, f0:512],
                        lhsT=kt[0:64, koff : koff + 128],
                        rhs=qt[0:64, f0:512],
                        start=True,
                        stop=not diag,
                        tile_position=(0, 0),
                    )
                    nc.tensor.matmul(
                        psv[:, 1, f0:512],
                        lhsT=kt[64:128, koff : koff + 128],
                        rhs=qt[64:128, f0:512],
                        start=True,
                        stop=not diag,
                        tile_position=(64, 0),
                    )
                    if diag:
                        # causal triangle: add MASK_BIAS above the diagonal
                        for g in range(2):
                            nc.tensor.matmul(
                                psv[:, g, f0 : f0 + 128],
                                lhsT=ident,
                                rhs=btri,
                                start=False,
                                stop=True,
                            )
                    et = epool.tile([128, 2, 512], F16, tag="e")
                    nc.scalar.activation(
                        out=et[:, :, f0:512],
                        in_=psv[:, :, f0:512],
                        func=mybir.ActivationFunctionType.Exp,
                        scale=SCALE,
                    )
                    ets[si] = et

                def emit_pv(si):
                    f0 = 512 - widths[si]
                    et = ets[si]
                    nc.tensor.matmul(
                        pv1[0:65, f0:512],
                        lhsT=V[si][:, h1, :],
                        rhs=et[:, 0, f0:512],
                        start=(si == 0),
                        stop=(si == nk - 1),
                    )
                    nc.tensor.matmul(
                        pv2[0:65, f0:512],
                        lhsT=V[si][:, h2, :],
                        rhs=et[:, 1, f0:512],
                        start=(si == 0),
                        stop=(si == nk - 1),
                    )

                emit_scores(0)
                emit_scores(1)
                for si in range(nk):
                    if si + 2 < nk:
                        emit_scores(si + 2)
                    # filler to cover exp(si) latency: ACT cost ~ 1.67*w + 140,
                    # PE cost this step ~ (w + 2*w)/2.4 -> deficit
                    w = widths[si]
                    deficit = int(1.67 * w + 140) - int(3 * w / 2.4)
                    take_filler(deficit)
                    if si == nk - 2 and mid_hook is not None:
                        mid_hook()
                    emit_pv(si)

                # ---- normalization: OCT = O / Z (staged via SBUF to free
                # the PV psum banks quickly for the next block) ----
                spv = spvpool.tile([128, 1024], F32, tag="spv")
                nc.vector.tensor_copy(spv[0:65, 0:512], pv1[0:65, :])
                nc.vector.tensor_copy(spv[0:65, 512:1024], pv2[0:65, :])
                rz = rzpool.tile([1, 1024], F32, tag="rz")
                nc.vector.tensor_copy(rz, spv[64:65, :])
                nc.vector.reciprocal_approx_fast(out=rz, in_=rz)
                rzb = rzpool.tile([64, 1024], F32, tag="rzb")
                nc.gpsimd.partition_broadcast(rzb, rz)
                oct_t = octpool.tile([128, 512], F16, tag=f"oct{pair}_{tcj}")
                OCT[pair][tcj] = oct_t
                nc.vector.tensor_mul(
                    oct_t[0:64, :], spv[0:64, 0:512], rzb[:, 0:512]
                )
                nc.vector.tensor_mul(
                    oct_t[64:128, :], spv[0:64, 512:1024], rzb[:, 512:1024]
                )

            # ---------- prologue: V(0..3), QK(tj=0) for pair 0 ----------
            for s in (0, 2):
                for _, emit in interleave(v_chunk_items(s), v_chunk_items(s + 1)):
                    emit()
            for _, emit in interleave(qk_chunk_items(0, 0), qk_chunk_items(0, 4)):
                emit()
            # pair p's block in round 0 needs qk(0, {p, 4+p}): feed them first
            for p in (1, 2, 3):
                filler.extend(
                    interleave(qk_chunk_items(0, p), qk_chunk_items(0, 4 + p))
                )

            # ---------- rounds ----------
            # round 0 filler: V(4..7) + qk(1); round 1: V(8..11)+qk(2)+proj(0..3)
            # round 2: V(12..15)+qk(3); round 3: proj(4..11)
            tail_state = {}

            def tail_mid_hook():
                # during the last block, pre-accumulate pairs 0..2 of the
                # t2=12,13 proj chunks into the (now free) scores psum tiles
                drain_filler()
                for t2 in (12, 13):
                    po = pspool.tile([128, 1024], F32, tag="ps", name="tailpo")
                    tail_state[t2] = po
                    pov = po.rearrange("p (g f) -> p g f", g=2)
                    for cj in range(2):
                        for p3 in range(3):
                            nc.tensor.matmul(
                                pov[:, cj, :],
                                lhsT=OCT[p3][3][:, 128 * (t2 % 4) : 128 * (t2 % 4) + 128],
                                rhs=wp[p3][:, 512 * cj : 512 * cj + 512],
                                start=(p3 == 0),
                                stop=False,
                            )

            tail_state["hook"] = tail_mid_hook

            for tcj in range(TJ):
                gated = 0  # leading deque items that gate later blocks this round
                if tcj == 0:
                    gated = len(filler)  # qk(0) pairs 1..3 pushed in prologue
                    s0 = 4
                    filler.extend(interleave(v_chunk_items(s0), v_chunk_items(s0 + 1)))
                    filler.extend(interleave(v_chunk_items(s0 + 2), v_chunk_items(s0 + 3)))
                    for n in range(4):
                        filler.extend(
                            interleave(qk_chunk_items(1, n), qk_chunk_items(1, n + 4))
                        )
                elif tcj == 1:
                    for s in (8, 10):
                        filler.extend(interleave(v_chunk_items(s), v_chunk_items(s + 1)))
                    for n in range(4):
                        filler.extend(
                            interleave(qk_chunk_items(2, n), qk_chunk_items(2, n + 4))
                        )
                elif tcj == 2:
                    for s in (12, 14):
                        filler.extend(interleave(v_chunk_items(s), v_chunk_items(s + 1)))
                    filler.extend(
                        interleave(qk_chunk_items(3, 0), qk_chunk_items(3, 4))
                    )
                    for t2 in range(0, 4):
                        filler.extend(
                            interleave(
                                proj_chunk_items(t2, 0), proj_chunk_items(t2, 1)
                            )
                        )
                else:
                    for p in (1, 2, 3):
                        filler.extend(
                            interleave(qk_chunk_items(3, p), qk_chunk_items(3, p + 4))
                        )
                    gated = 48
                    for t2 in range(4, 12):
                        filler.extend(
                            interleave(
                                proj_chunk_items(t2, 0), proj_chunk_items(t2, 1)
                            )
                        )

                rt_total = len(filler)
                for pair in range(PAIRS):
                    if gated and pair > 0:
                        # qk chunks for this pair must be emitted before its
                        # block reads QT/KT: first 16*pair gated deque items
                        need = 16 * pair - (rt_total - len(filler))
                        if need > 0:
                            drain_filler(need)
                    hook = tail_state["hook"] if (tcj == 3 and pair == 3) else None
                    attention_block(pair, tcj, mid_hook=hook)
                drain_filler()

            # ---------- tail: pair-3 matmuls of t2 12,13 then t2 14,15 ----------
            for t2 in (12, 13):
                po = tail_state[t2]
                pov = po.rearrange("p (g f) -> p g f", g=2)
                for cj in range(2):
                    nc.tensor.matmul(
                        pov[:, cj, :],
                        lhsT=OCT[3][3][:, 128 * (t2 % 4) : 128 * (t2 % 4) + 128],
                        rhs=wp[3][:, 512 * cj : 512 * cj + 512],
                        start=False,
                        stop=True,
                    )
                    ot = opool.tile([128, 512], F16, tag="out", name="ott")
                    nc.scalar.copy(ot, pov[:, cj, :])
                    nc.gpsimd.dma_start(
                        out=out_d[128 * t2 : 128 * t2 + 128, 512 * cj : 512 * cj + 512],
                        in_=ot,
                    )
            for cj in range(2):
                for _, emit in interleave(
                    proj_chunk_items(14, cj, evict_on_act=True),
                    proj_chunk_items(15, cj, evict_on_act=True),
                ):
                    emit()

    nc.finalize()
    return nc


def _get_nc():
    if "nc" not in _CACHE:
        _CACHE["nc"] = _build()
    return _CACHE["nc"]


def kernel(x, W_qkv, b_qkv, W_proj, b_proj):
    from concourse.bass_utils import run_bass_kernel_spmd

    x = np.asarray(x, dtype=np.float32)
    W_qkv = np.asarray(W_qkv, dtype=np.float32)
    b_qkv = np.asarray(b_qkv, dtype=np.float32)
    W_proj = np.asarray(W_proj, dtype=np.float32)
    b_proj = np.asarray(b_proj, dtype=np.float32)

    in_maps = []
    for core in range(N_CORES):
        b = core // 2
        h2 = core % 2
        o = 512 * h2
        xt = np.ascontiguousarray(x[b].T).astype(np.float16)
        wq = W_qkv[:, o : o + 512]
        wk = W_qkv[:, 1024 + o : 1024 + o + 512]
        wqk = np.ascontiguousarray(np.concatenate([wq, wk], axis=1)).astype(np.float16)
        wv = np.ascontiguousarray(W_qkv[:, 2048 + o : 2048 + o + 512]).astype(np.float16)
        wp = np.ascontiguousarray(W_proj[o : o + 512, :]).astype(np.float16)
        bq = b_qkv[o : o + 512]
        bk = b_qkv[1024 + o : 1024 + o + 512]
        bqk = np.ascontiguousarray(np.concatenate([bq, bk]).reshape(8, 128).T)
        bv = np.ascontiguousarray(
            b_qkv[2048 + o : 2048 + o + 512].reshape(1, 512)
        ).astype(np.float16)
        in_maps.append(
            {"xt": xt, "wqk": wqk, "wv": wv, "wp": wp, "bqk": bqk, "bv": bv}
        )

    nc = _get_nc()
    kwargs = {}
    if os.environ.get("BASS_KERNEL_TRACE"):
        kwargs["trace"] = True
    res = run_bass_kernel_spmd(nc, in_maps, core_ids=list(range(N_CORES)), **kwargs)
    _CACHE["last_results"] = res

    out = np.empty((B, T, N_EMBD), dtype=np.float32)
    for b in range(B):
        out[b] = (
            res.results[2 * b]["out_p"].astype(np.float32)
            + res.results[2 * b + 1]["out_p"].astype(np.float32)
            + b_proj[None, :]
        )
    return out
